# revision 8
# baseline (speedup 1.0000x reference)
"""Tensor-parallel MultiHeadAttention (QKV + RoPE + GQA causal SDPA + dense)
for 8 Trainium2 NeuronCores.

Sharding (TP): core d owns query heads {2d, 2d+1} and kv head d//2 (kv heads
replicated across core pairs), plus the matching 256 rows of w_dense. Each
core produces a full-shape [S, E] partial; the all-reduce is a host-side sum
over the 8 fp16 partials.

All matmul operands are fp16 (1.0 PE cycles/row, same as bf16; half the DMA
bytes of the old float32r kernel); PSUM accumulation stays fp32, so the end
to-end rel-err is ~1e-3 against the 2e-2 gate.

Per-core pipeline, two phases:

Phase A (QKV + RoPE + v transpose), sequential over the 4 fused-row blocks
(q0, q1, k, v) per 512-query chunk so each block's 16-matmul contraction
chain owns one PSUM bank and the post-processing of block f overlaps the
chain of block f+1 (rot matmuls are emitted one chain late to hide the
PSUM->SBUF copy latency from the in-order PE queue):
  qkv^T = W_f^T x^T -> [128, 512] ; RoPE via permutation matmul + DVE
  combine (all-fp16 DVE ops take the 2x path); v transposed 128x128 via PE.

Phase B (attention + dense), per 512-query chunk. Key tiles are processed in
PAIRS: the two S^T = k^T q matmuls of a pair land in one 2-bank PSUM tile so
one ScalarE exp covers 1024 columns (ScalarE is the attention-phase
bottleneck; wider activations amortize its ~217ns/instr overhead). The two
heads' streams interleave and ctx matmuls lag one pair-step behind the S
matmuls, so the in-order PE queue never waits on an exp. Diagonal pairs
compute/exp only their causally visible column ranges (no wasted exp work,
no stale-PSUM reads); the partial 128-wide triangles are masked
multiplicatively after exp. Softmax denominators accumulate on DVE in fp16,
are column-summed with a ones-vector matmul, reciprocal'd, and broadcast.
Dense matmuls of chunk c-1 are interleaved as PE filler inside chunk c's
attention steps; their PSUM->SBUF fp16 drains round-robin over ScalarE/DVE
(GPSIMD and DMA have no PSUM route).
"""

import collections

import numpy as np

B, S, E = 1, 2048, 2048
H, KVH, D = 16, 4, 128
NCORES = 8
P = 128
FD = 512            # matmul moving free dim == one fp32 PSUM bank
NE = E // P         # 16 contraction tiles over the embedding dim
NSC = S // FD       # 4 sequence chunks
NST = S // P        # 16 sequence tiles
FLOC = 4 * P        # local fused qkv rows per core (2 q heads + k + v)
ROPE_BASE = 10000.0
# causally visible query sub-range start for diagonal sk tile o
DIAG_START = (0, 128, 256, 384)

LAST_RESULT = None
_BASS_CACHE = None


def _rope_tables():
    inv = 1.0 / (ROPE_BASE ** (np.arange(0, D, 2, dtype=np.float64) / D))
    t = np.arange(S, dtype=np.float64)
    freqs = np.outer(t, inv)
    emb = np.concatenate([freqs, freqs], axis=-1)  # [S, D]
    return np.cos(emb), np.sin(emb)


def _host_constants():
    cos, sin = _rope_tables()
    consts = {}
    consts["cosr"] = np.ascontiguousarray(cos.T).astype(np.float16)
    consts["sinr"] = np.ascontiguousarray(sin.T).astype(np.float16)
    r_idx = np.arange(P)[:, None]
    c_idx = np.arange(P)[None, :]
    consts["tri"] = (r_idx <= c_idx).astype(np.float16)
    # rotate_half as a matmul: rot = M @ q (in [d, s] layout); pass M.T as lhsT
    M = np.zeros((P, P), np.float16)
    half = D // 2
    M[np.arange(half), np.arange(half) + half] = -1.0
    M[np.arange(half) + half, np.arange(half)] = 1.0
    consts["protT"] = np.ascontiguousarray(M.T)
    consts["ident"] = np.eye(P, dtype=np.float16)
    consts["ones"] = np.ones((P, 1), np.float16)
    return consts


def _build_bass():
    import concourse.mybir as mybir
    import concourse.tile as tile
    from concourse import bacc

    f32 = mybir.dt.float32
    f16 = mybir.dt.float16
    Exp = mybir.ActivationFunctionType.Exp

    nc = bacc.Bacc(None, target_bir_lowering=False, name="mha_tp8v2")
    xTt = nc.dram_tensor("xTt", [NE, P, S], f16, kind="ExternalInput")
    wqkvT = nc.dram_tensor("wqkvT", [4, P, 4, FLOC], f16, kind="ExternalInput")
    wdT = nc.dram_tensor("wdT", [P, 2, S], f16, kind="ExternalInput")
    cosr = nc.dram_tensor("cosr", [P, S], f16, kind="ExternalInput")
    sinr = nc.dram_tensor("sinr", [P, S], f16, kind="ExternalInput")
    trid = nc.dram_tensor("tri", [P, P], f16, kind="ExternalInput")
    protT = nc.dram_tensor("protT", [P, P], f16, kind="ExternalInput")
    ident = nc.dram_tensor("ident", [P, P], f16, kind="ExternalInput")
    ones = nc.dram_tensor("ones", [P, 1], f16, kind="ExternalInput")
    # output tiled [c, st, eo, p, f]; host reassembles to [s, e]
    out = nc.dram_tensor("out", [NSC, 4, 2, P, 2 * FD], f16, kind="ExternalOutput")

    with tile.TileContext(nc) as tc:
        with tc.tile_pool(name="const", bufs=1) as const:
            xs = const.tile([P, NE, S], f16, name="xs")
            w_sb = const.tile([P, NE, FLOC], f16, name="w_sb")
            wd_sb = const.tile([P, 2, S], f16, name="wd_sb")
            cq = const.tile([P, S], f16, name="cq")
            sq_t = const.tile([P, S], f16, name="sq_t")
            mk = const.tile([P, P], f16, name="mk")
            pr = const.tile([P, P], f16, name="pr")
            idn = const.tile([P, P], f16, name="idn")
            on = const.tile([P, 1], f16, name="on")

            junk = const.tile([P, P], f16, name="junk")
            qr = const.tile([P, 2, S], f16, name="qr")
            kr = const.tile([P, S], f16, name="kr")
            vn = const.tile([P, NST, P], f16, name="vn")

            # ---- Phase A: fused QKV projection + RoPE + v transpose ----
            with tc.tile_pool(name="pq", bufs=1, space="PSUM") as pqkv, \
                 tc.tile_pool(name="prt", bufs=2, space="PSUM") as prot_p, \
                 tc.tile_pool(name="pv", bufs=1, space="PSUM") as pvt, \
                 tc.tile_pool(name="pw", bufs=1, space="PSUM") as pwarm, \
                 tc.tile_pool(name="rtmp", bufs=4) as rtmp, \
                 tc.tile_pool(name="vtp", bufs=2) as vtp:
                # PE warm-up on a memset tile (no DMA dependency): ramps the
                # Tensor engine to max p-state while weights/x are in flight
                nc.vector.memset(junk, 1.0)
                warm = pwarm.tile([1, P], f32, tag="warm", name="warm")
                for _ in range(40):
                    nc.tensor.matmul(warm, lhsT=junk[:, 0:1], rhs=junk,
                                     start=True, stop=True)
                # consts on the gpsimd SWDGE ring (not start-critical)
                nc.gpsimd.dma_start(pr, protT[:, :])
                nc.gpsimd.dma_start(cq, cosr[:, :])
                nc.gpsimd.dma_start(sq_t, sinr[:, :])
                nc.gpsimd.dma_start(idn, ident[:, :])
                nc.gpsimd.dma_start(on, ones[:, :])
                nc.gpsimd.dma_start(mk, trid[:, :])
                nc.gpsimd.dma_start(wd_sb, wdT[:, :, :])
                # w first (4-eo groups, 4KB descriptors), then per-eo x slabs
                # (4KB descriptors) 3-way across the sync/scalar/gpsimd rings:
                # ring descriptor GENERATION (~8ns/desc) is the real input
                # pacer, so balance descriptor count, not bytes
                nc.sync.dma_start(w_sb[:, 0:4, :], wqkvT[0])
                nc.scalar.dma_start(w_sb[:, 4:8, :], wqkvT[1])
                nc.sync.dma_start(w_sb[:, 8:12, :], wqkvT[2])
                nc.scalar.dma_start(w_sb[:, 12:16, :], wqkvT[3])
                for eo in range(NE):
                    ring = nc.sync if eo % 2 == 0 else nc.scalar
                    ring.dma_start(xs[:, eo, :], xTt[eo])

                def make_rope(c, f, qt, ssl):
                    def emit():
                        rps = prot_p.tile([P, FD], f32, tag="rot",
                                          name=f"rot_{c}_{f}")
                        nc.tensor.matmul(rps, lhsT=pr, rhs=qt,
                                         start=True, stop=True)
                        tt = rtmp.tile([P, FD], f16, tag="tt",
                                       name=f"tt_{c}_{f}")
                        nc.vector.tensor_mul(tt, rps, sq_t[:, ssl])
                        dst = qr[:, f, ssl] if f < 2 else kr[:, ssl]
                        nc.vector.tensor_mul(dst, qt, cq[:, ssl])
                        nc.vector.tensor_add(dst, dst, tt)
                    return emit

                def make_vt(c, vtt):
                    def emit():
                        vp = pvt.tile([P, 4, P], f16, tag="vt",
                                      name=f"vt_{c}")
                        for jj in range(4):
                            nc.tensor.transpose(
                                vp[:, jj, :], vtt[:, jj * P:(jj + 1) * P],
                                idn)
                        nc.scalar.copy(vn[:, 4 * c:4 * c + 4, :], vp)
                    return emit

                pend = collections.deque()
                for c in range(NSC):
                    ssl = slice(c * FD, (c + 1) * FD)
                    psums = [
                        pqkv.tile([P, FD], f32, tag=f"qkv{f}",
                                  name=f"qkv_{c}_{f}")
                        for f in range(4)
                    ]
                    for eo in range(NE):
                        for f in range(4):
                            nc.tensor.matmul(
                                psums[f],
                                lhsT=w_sb[:, eo, f * P:(f + 1) * P],
                                rhs=xs[:, eo, ssl],
                                start=(eo == 0),
                                stop=(eo == NE - 1),
                            )
                        # previous chunk's rope/vt PE work interleaves here
                        # (the eo==0 pop also hides the qkv-psum copy wait at
                        # each chunk boundary)
                        if (eo == 0 or eo % 4 == 3) and pend:
                            pend.popleft()()
                    for f in range(4):
                        if f < 3:
                            qt = rtmp.tile([P, FD], f16, tag=f"qt{f}",
                                           name=f"qt_{c}_{f}")
                            nc.scalar.copy(qt, psums[f])
                            pend.append(make_rope(c, f, qt, ssl))
                        else:
                            vtt = vtp.tile([P, FD], f16, tag="vT",
                                           name=f"vT_{c}")
                            nc.scalar.copy(vtt, psums[f])
                            pend.append(make_vt(c, vtt))
                while pend:
                    pend.popleft()()

            # ---- Phase B: attention + dense, per 512-query chunk ----
            with tc.tile_pool(name="ps_s", bufs=1, space="PSUM") as ps_s, \
                 tc.tile_pool(name="ps_ctx", bufs=1, space="PSUM") as ps_ctx, \
                 tc.tile_pool(name="ps_o", bufs=2, space="PSUM") as ps_o, \
                 tc.tile_pool(name="ptp", bufs=3) as ptp, \
                 tc.tile_pool(name="accp", bufs=2) as accp, \
                 tc.tile_pool(name="rbp", bufs=2) as rbp, \
                 tc.tile_pool(name="ctp", bufs=2) as ctp, \
                 tc.tile_pool(name="outp", bufs=6) as outp:

                ct_tiles = {}
                filler = collections.deque()
                copy_rr = [0]

                ot_hold = {}

                def emit_dense_unit(c, st, eo):
                    op = ps_o.tile([P, FD], f32, tag="dop",
                                   name=f"dop_{c}_{st}_{eo}")
                    nc.tensor.matmul(
                        op, lhsT=ct_tiles[(c, 0)][:, st * P:(st + 1) * P],
                        rhs=wd_sb[:, 0, eo * FD:(eo + 1) * FD],
                        start=True, stop=False)
                    nc.tensor.matmul(
                        op, lhsT=ct_tiles[(c, 1)][:, st * P:(st + 1) * P],
                        rhs=wd_sb[:, 1, eo * FD:(eo + 1) * FD],
                        start=False, stop=True)
                    # eo pairs share one [P, 1024] staging tile so the out
                    # DMA moves 2KB per partition line (ring-gen efficiency)
                    ep = eo // 2
                    if eo % 2 == 0:
                        ot_hold[(c, st, ep)] = outp.tile(
                            [P, 2 * FD], f16, tag="ot",
                            name=f"ot_{c}_{st}_{ep}")
                    ot = ot_hold[(c, st, ep)]
                    half = slice((eo % 2) * FD, (eo % 2) * FD + FD)
                    i = copy_rr[0]
                    copy_rr[0] += 1
                    if i % 2 == 0:
                        nc.scalar.copy(ot[:, half], op)
                    else:
                        nc.vector.tensor_copy(ot[:, half], op)
                    if eo % 2 == 1:
                        ring = nc.scalar if st == 3 else nc.sync
                        ring.dma_start(out[c, st, ep], ot)

                def pop_fillers(k):
                    for _ in range(k):
                        if filler:
                            filler.popleft()()

                def emit_attn(c):
                    nj = 4 * c + 4
                    npair = nj // 2
                    qbase = c * FD
                    ctxps = {
                        h: ps_ctx.tile([P, FD], f32, tag=f"ctx{h}",
                                       name=f"ctx_{c}_{h}")
                        for h in (0, 1)
                    }
                    acc2 = {
                        h: accp.tile([P, 2 * FD], f16, tag=f"acc{h}",
                                     name=f"acc_{c}_{h}")
                        for h in (0, 1)
                    }
                    pt_hist = {}

                    def emit_C(J):
                        for h in (0, 1):
                            pt2 = pt_hist[(h, J)]
                            for t in (0, 1):
                                j = 2 * J + t
                                o = j - 4 * c
                                so = DIAG_START[o] if o >= 0 else 0
                                nc.tensor.matmul(
                                    ctxps[h][:, so:],
                                    lhsT=vn[:, j, :],
                                    rhs=pt2[:, t * FD + so:(t + 1) * FD],
                                    start=(j == 0), stop=(j == nj - 1),
                                )

                    for J in range(npair):
                        diagA = J == npair - 2
                        diagB = J == npair - 1
                        for h in (0, 1):
                            sp2 = ps_s.tile([P, 2 * FD], f32, tag=f"s{h}",
                                            name=f"s_{c}_{h}_{J}")
                            for t in (0, 1):
                                j = 2 * J + t
                                o = j - 4 * c
                                so = DIAG_START[o] if o >= 0 else 0
                                nc.tensor.matmul(
                                    sp2[:, t * FD + so:(t + 1) * FD],
                                    lhsT=kr[:, j * P:(j + 1) * P],
                                    rhs=qr[:, h, qbase + so:qbase + FD],
                                    start=True, stop=True,
                                )
                            pt2 = ptp.tile([P, 2 * FD], f16, tag=f"pt{h}",
                                           name=f"pt_{c}_{h}_{J}")
                            if diagA:
                                rngs = ((0, FD), (FD + 128, 2 * FD))
                            elif diagB:
                                rngs = ((256, FD), (FD + 384, 2 * FD))
                            else:
                                rngs = ((0, 2 * FD),)
                            for a, b in rngs:
                                nc.scalar.activation(pt2[:, a:b], sp2[:, a:b],
                                                     Exp)
                            if diagA:
                                nc.gpsimd.tensor_mul(
                                    pt2[:, 0:128], pt2[:, 0:128], mk)
                                nc.gpsimd.tensor_mul(
                                    pt2[:, FD + 128:FD + 256],
                                    pt2[:, FD + 128:FD + 256], mk)
                            if diagB:
                                nc.gpsimd.tensor_mul(
                                    pt2[:, 256:384], pt2[:, 256:384], mk)
                                nc.gpsimd.tensor_mul(
                                    pt2[:, FD + 384:2 * FD],
                                    pt2[:, FD + 384:2 * FD], mk)
                            if J == 0:
                                if c == 0:
                                    nc.vector.memset(
                                        acc2[h][:, FD:FD + 128], 0.0)
                                    for a, b in rngs:
                                        nc.vector.tensor_copy(
                                            acc2[h][:, a:b], pt2[:, a:b])
                                else:
                                    nc.vector.tensor_copy(acc2[h], pt2)
                            else:
                                for a, b in rngs:
                                    nc.vector.tensor_add(
                                        acc2[h][:, a:b], acc2[h][:, a:b],
                                        pt2[:, a:b])
                            pt_hist[(h, J)] = pt2
                        if J >= 1:
                            emit_C(J - 1)
                        pop_fillers(3)
                    emit_C(npair - 1)
                    # per-head softmax tail
                    for h in (0, 1):
                        rp_t = ps_o.tile([P, FD], f32, tag="dop",
                                         name=f"rp_{c}_{h}")
                        rp = rp_t[0:1, 0:FD]
                        nc.tensor.matmul(rp, lhsT=on, rhs=acc2[h][:, :FD],
                                         start=True, stop=False)
                        nc.tensor.matmul(rp, lhsT=on, rhs=acc2[h][:, FD:],
                                         start=False, stop=True)
                        rec = rbp.tile([1, FD], f32, tag="rec",
                                       name=f"rec_{c}_{h}")
                        nc.vector.reciprocal_approx_fast(rec, rp)
                        rb = rbp.tile([P, FD], f32, tag="rb",
                                      name=f"rb_{c}_{h}")
                        nc.gpsimd.partition_broadcast(rb, rec)
                        ct = ctp.tile([P, FD], f16, tag=f"ct{h}",
                                      name=f"ct_{c}_{h}")
                        nc.vector.tensor_mul(ct, ctxps[h], rb)
                        ct_tiles[(c, h)] = ct
                        pop_fillers(1)

                for c in range(NSC):
                    emit_attn(c)
                    for st in range(4):
                        for eo in range(4):
                            filler.append(
                                lambda c=c, st=st, eo=eo:
                                emit_dense_unit(c, st, eo))
                while filler:
                    filler.popleft()()
    nc.compile()
    return nc


def make_in_maps(x, w_qkv, w_dense):
    x = np.asarray(x, np.float32).reshape(S, E)
    w_qkv = np.asarray(w_qkv, np.float32)
    w_dense = np.asarray(w_dense, np.float32)
    xTt = np.ascontiguousarray(x.T.reshape(NE, P, S)).astype(np.float16)
    consts = _host_constants()
    in_maps = []
    scale = np.float32(1.0 / np.sqrt(D))
    for d in range(NCORES):
        g = d // 2
        wq = w_qkv[2 * d * P:(2 * d + 2) * P] * scale
        wk = w_qkv[H * D + g * P: H * D + (g + 1) * P]
        wv = w_qkv[H * D + KVH * D + g * P: H * D + KVH * D + (g + 1) * P]
        wcat = np.concatenate([wq, wk, wv], 0)          # [FLOC, E]
        # [4-eo-group, p, eo-in-group, floc]: 4KB contiguous per partition
        wqkvT_d = np.ascontiguousarray(
            wcat.T.reshape(4, 4, P, FLOC).transpose(0, 2, 1, 3)
        ).astype(np.float16)
        wdT_d = np.ascontiguousarray(
            w_dense[:, 2 * d * P:(2 * d + 2) * P].T
            .reshape(2, P, E).transpose(1, 0, 2)).astype(np.float16)
        m = {"xTt": xTt, "wqkvT": wqkvT_d, "wdT": wdT_d}
        m.update(consts)
        in_maps.append(m)
    return in_maps


def kernel(x, w_qkv, w_dense):
    global LAST_RESULT, _BASS_CACHE
    from concourse.bass_utils import run_bass_kernel_spmd

    in_maps = make_in_maps(x, w_qkv, w_dense)
    if _BASS_CACHE is None:
        _BASS_CACHE = _build_bass()
    res = run_bass_kernel_spmd(_BASS_CACHE, in_maps, core_ids=list(range(NCORES)))
    LAST_RESULT = res
    # sum partials over cores, then untile [c, st, eo, p, f] -> [s, e]
    acc = np.zeros((NSC, 4, 2, P, 2 * FD), np.float32)
    for r in res.results:
        acc += r["out"].astype(np.float32)
    full = acc.transpose(0, 1, 3, 2, 4).reshape(S, E)
    return np.ascontiguousarray(full).reshape(B, S, E)


# revision 10
# speedup vs baseline: 1.2800x; 1.2800x over previous
"""Tensor-parallel MultiHeadAttention (QKV + RoPE + GQA causal SDPA + dense)
for 8 Trainium2 NeuronCores.

Sharding (TP): core d owns query heads {2d, 2d+1} and kv head d//2 (kv heads
replicated across core pairs), plus the matching 256 rows of w_dense. Each
core produces a full-shape [S, E] partial; the all-reduce is a host-side sum
over the 8 fp16 partials.

All matmul operands are fp16 (1.0 PE cycles/row, same as bf16; half the DMA
bytes of the old float32r kernel); PSUM accumulation stays fp32, so the end
to-end rel-err is ~1e-3 against the 2e-2 gate.

Per-core pipeline, two phases:

Phase A (QKV + RoPE + v transpose), sequential over the 4 fused-row blocks
(q0, q1, k, v) per 512-query chunk so each block's 16-matmul contraction
chain owns one PSUM bank and the post-processing of block f overlaps the
chain of block f+1 (rot matmuls are emitted one chain late to hide the
PSUM->SBUF copy latency from the in-order PE queue):
  qkv^T = W_f^T x^T -> [128, 512] ; RoPE via permutation matmul + DVE
  combine (all-fp16 DVE ops take the 2x path); v transposed 128x128 via PE.

Phase B (attention + dense), per 512-query chunk. Key tiles are processed in
PAIRS: the two S^T = k^T q matmuls of a pair land in one 2-bank PSUM tile so
one ScalarE exp covers 1024 columns (ScalarE is the attention-phase
bottleneck; wider activations amortize its ~217ns/instr overhead). The two
heads' streams interleave and ctx matmuls lag one pair-step behind the S
matmuls, so the in-order PE queue never waits on an exp. Diagonal pairs
compute/exp only their causally visible column ranges (no wasted exp work,
no stale-PSUM reads); the partial 128-wide triangles are masked
multiplicatively after exp. Softmax denominators accumulate on DVE in fp16,
are column-summed with a ones-vector matmul, reciprocal'd, and broadcast.
Dense matmuls of chunk c-1 are interleaved as PE filler inside chunk c's
attention steps; their PSUM->SBUF fp16 drains round-robin over ScalarE/DVE
(GPSIMD and DMA have no PSUM route).
"""

import collections

import numpy as np

B, S, E = 1, 2048, 2048
H, KVH, D = 16, 4, 128
NCORES = 8
P = 128
FD = 512            # matmul moving free dim == one fp32 PSUM bank
NE = E // P         # 16 contraction tiles over the embedding dim
NSC = S // FD       # 4 sequence chunks
NST = S // P        # 16 sequence tiles
FLOC = 4 * P        # local fused qkv rows per core (2 q heads + k + v)
ROPE_BASE = 10000.0
# causally visible query sub-range start for diagonal sk tile o
DIAG_START = (0, 128, 256, 384)

LAST_RESULT = None
_BASS_CACHE = None


def _rope_tables():
    inv = 1.0 / (ROPE_BASE ** (np.arange(0, D, 2, dtype=np.float64) / D))
    t = np.arange(S, dtype=np.float64)
    freqs = np.outer(t, inv)
    emb = np.concatenate([freqs, freqs], axis=-1)  # [S, D]
    return np.cos(emb), np.sin(emb)


def _host_constants():
    cos, sin = _rope_tables()
    consts = {}
    consts["cosr"] = np.ascontiguousarray(cos.T).astype(np.float16)
    consts["sinr"] = np.ascontiguousarray(sin.T).astype(np.float16)
    r_idx = np.arange(P)[:, None]
    c_idx = np.arange(P)[None, :]
    consts["tri"] = (r_idx <= c_idx).astype(np.float16)
    # rotate_half as a matmul: rot = M @ q (in [d, s] layout); pass M.T as lhsT
    M = np.zeros((P, P), np.float16)
    half = D // 2
    M[np.arange(half), np.arange(half) + half] = -1.0
    M[np.arange(half) + half, np.arange(half)] = 1.0
    consts["protT"] = np.ascontiguousarray(M.T)
    consts["ident"] = np.eye(P, dtype=np.float16)
    consts["ones"] = np.ones((P, 1), np.float16)
    return consts


def _build_bass():
    import concourse.mybir as mybir
    import concourse.tile as tile
    from concourse import bacc

    f32 = mybir.dt.float32
    f16 = mybir.dt.float16
    Exp = mybir.ActivationFunctionType.Exp

    nc = bacc.Bacc(None, target_bir_lowering=False, name="mha_tp8v2")
    xTt = nc.dram_tensor("xTt", [NE, P, S], f16, kind="ExternalInput")
    wqkvT = nc.dram_tensor("wqkvT", [4, P, 4, FLOC], f16, kind="ExternalInput")
    wdT = nc.dram_tensor("wdT", [P, 2, S], f16, kind="ExternalInput")
    cosr = nc.dram_tensor("cosr", [P, S], f16, kind="ExternalInput")
    sinr = nc.dram_tensor("sinr", [P, S], f16, kind="ExternalInput")
    trid = nc.dram_tensor("tri", [P, P], f16, kind="ExternalInput")
    protT = nc.dram_tensor("protT", [P, P], f16, kind="ExternalInput")
    ident = nc.dram_tensor("ident", [P, P], f16, kind="ExternalInput")
    ones = nc.dram_tensor("ones", [P, 1], f16, kind="ExternalInput")
    # output tiled [c, st, eo, p, f]; host reassembles to [s, e]
    out = nc.dram_tensor("out", [NSC, 4, 2, P, 2 * FD], f16, kind="ExternalOutput")

    with tile.TileContext(nc) as tc:
        with tc.tile_pool(name="const", bufs=1) as const:
            xs = const.tile([P, NE, S], f16, name="xs")
            w_sb = const.tile([P, NE, FLOC], f16, name="w_sb")
            wd_sb = const.tile([P, 2, S], f16, name="wd_sb")
            cq = const.tile([P, S], f16, name="cq")
            sq_t = const.tile([P, S], f16, name="sq_t")
            mk = const.tile([P, P], f16, name="mk")
            pr = const.tile([P, P], f16, name="pr")
            idn = const.tile([P, P], f16, name="idn")
            on = const.tile([P, 1], f16, name="on")

            junk = const.tile([P, P], f16, name="junk")
            qr = const.tile([P, 2, S], f16, name="qr")
            kr = const.tile([P, S], f16, name="kr")
            vn = const.tile([P, NST, P], f16, name="vn")

            # ---- Phase A: fused QKV projection + RoPE + v transpose ----
            with tc.tile_pool(name="pq", bufs=1, space="PSUM") as pqkv, \
                 tc.tile_pool(name="prt", bufs=2, space="PSUM") as prot_p, \
                 tc.tile_pool(name="pv", bufs=1, space="PSUM") as pvt, \
                 tc.tile_pool(name="pw", bufs=1, space="PSUM") as pwarm, \
                 tc.tile_pool(name="rtmp", bufs=4) as rtmp, \
                 tc.tile_pool(name="vtp", bufs=2) as vtp:
                # PE warm-up on a memset tile (no DMA dependency): ramps the
                # Tensor engine to max p-state while weights/x are in flight
                nc.vector.memset(junk, 1.0)
                warm = pwarm.tile([1, P], f32, tag="warm", name="warm")
                for _ in range(40):
                    nc.tensor.matmul(warm, lhsT=junk[:, 0:1], rhs=junk,
                                     start=True, stop=True)
                # consts on the gpsimd SWDGE ring (not start-critical)
                nc.gpsimd.dma_start(pr, protT[:, :])
                nc.gpsimd.dma_start(cq, cosr[:, :])
                nc.gpsimd.dma_start(sq_t, sinr[:, :])
                nc.gpsimd.dma_start(idn, ident[:, :])
                nc.gpsimd.dma_start(on, ones[:, :])
                nc.gpsimd.dma_start(mk, trid[:, :])
                nc.gpsimd.dma_start(wd_sb, wdT[:, :, :])
                # w first (4-eo groups, 4KB descriptors), then per-eo x slabs
                # (4KB descriptors) 3-way across the sync/scalar/gpsimd rings:
                # ring descriptor GENERATION (~8ns/desc) is the real input
                # pacer, so balance descriptor count, not bytes
                nc.sync.dma_start(w_sb[:, 0:4, :], wqkvT[0])
                nc.scalar.dma_start(w_sb[:, 4:8, :], wqkvT[1])
                nc.sync.dma_start(w_sb[:, 8:12, :], wqkvT[2])
                nc.scalar.dma_start(w_sb[:, 12:16, :], wqkvT[3])
                nc.sync.dma_start(xs[:, 0, 0:FD], xTt[0, :, 0:FD])
                nc.scalar.dma_start(xs[:, 0, FD:], xTt[0, :, FD:])
                for eo in range(1, NE):
                    ring = nc.sync if eo % 2 == 0 else nc.scalar
                    ring.dma_start(xs[:, eo, :], xTt[eo])

                def make_rope(c, f, qt, ssl):
                    def emit():
                        rps = prot_p.tile([P, FD], f32, tag="rot",
                                          name=f"rot_{c}_{f}")
                        nc.tensor.matmul(rps, lhsT=pr, rhs=qt,
                                         start=True, stop=True)
                        tt = rtmp.tile([P, FD], f16, tag="tt",
                                       name=f"tt_{c}_{f}")
                        nc.vector.tensor_mul(tt, rps, sq_t[:, ssl])
                        dst = qr[:, f, ssl] if f < 2 else kr[:, ssl]
                        nc.vector.tensor_mul(dst, qt, cq[:, ssl])
                        nc.vector.tensor_add(dst, dst, tt)
                    return emit

                def make_vt(c, vtt):
                    def emit():
                        vp = pvt.tile([P, 4, P], f16, tag="vt",
                                      name=f"vt_{c}")
                        for jj in range(4):
                            nc.tensor.transpose(
                                vp[:, jj, :], vtt[:, jj * P:(jj + 1) * P],
                                idn)
                        nc.scalar.copy(vn[:, 4 * c:4 * c + 4, :], vp)
                    return emit

                pend = collections.deque()
                for c in range(NSC):
                    ssl = slice(c * FD, (c + 1) * FD)
                    psums = [
                        pqkv.tile([P, FD], f32, tag=f"qkv{f}",
                                  name=f"qkv_{c}_{f}")
                        for f in range(4)
                    ]
                    for eo in range(NE):
                        for f in range(4):
                            nc.tensor.matmul(
                                psums[f],
                                lhsT=w_sb[:, eo, f * P:(f + 1) * P],
                                rhs=xs[:, eo, ssl],
                                start=(eo == 0),
                                stop=(eo == NE - 1),
                            )
                        # previous chunk's rope/vt PE work interleaves here
                        # (the eo==0 pop also hides the qkv-psum copy wait at
                        # each chunk boundary)
                        if (eo == 0 or eo % 4 == 3) and pend:
                            pend.popleft()()
                    for f in range(4):
                        if f < 3:
                            qt = rtmp.tile([P, FD], f16, tag=f"qt{f}",
                                           name=f"qt_{c}_{f}")
                            nc.scalar.copy(qt, psums[f])
                            pend.append(make_rope(c, f, qt, ssl))
                        else:
                            vtt = vtp.tile([P, FD], f16, tag="vT",
                                           name=f"vT_{c}")
                            nc.scalar.copy(vtt, psums[f])
                            pend.append(make_vt(c, vtt))
                while pend:
                    pend.popleft()()

            # ---- Phase B: attention + dense, per 512-query chunk ----
            with tc.tile_pool(name="ps_s", bufs=1, space="PSUM") as ps_s, \
                 tc.tile_pool(name="ps_ctx", bufs=1, space="PSUM") as ps_ctx, \
                 tc.tile_pool(name="ps_o", bufs=2, space="PSUM") as ps_o, \
                 tc.tile_pool(name="ptp", bufs=3) as ptp, \
                 tc.tile_pool(name="accp", bufs=2) as accp, \
                 tc.tile_pool(name="rbp", bufs=2) as rbp, \
                 tc.tile_pool(name="ctp", bufs=2) as ctp, \
                 tc.tile_pool(name="outp", bufs=6) as outp:

                ct_tiles = {}
                filler = collections.deque()
                copy_rr = [0]

                ot_hold = {}

                def emit_dense_unit(c, st, eo):
                    op = ps_o.tile([P, FD], f32, tag="dop",
                                   name=f"dop_{c}_{st}_{eo}")
                    nc.tensor.matmul(
                        op, lhsT=ct_tiles[(c, 0)][:, st * P:(st + 1) * P],
                        rhs=wd_sb[:, 0, eo * FD:(eo + 1) * FD],
                        start=True, stop=False)
                    nc.tensor.matmul(
                        op, lhsT=ct_tiles[(c, 1)][:, st * P:(st + 1) * P],
                        rhs=wd_sb[:, 1, eo * FD:(eo + 1) * FD],
                        start=False, stop=True)
                    # eo pairs share one [P, 1024] staging tile so the out
                    # DMA moves 2KB per partition line (ring-gen efficiency)
                    ep = eo // 2
                    if eo % 2 == 0:
                        ot_hold[(c, st, ep)] = outp.tile(
                            [P, 2 * FD], f16, tag="ot",
                            name=f"ot_{c}_{st}_{ep}")
                    ot = ot_hold[(c, st, ep)]
                    half = slice((eo % 2) * FD, (eo % 2) * FD + FD)
                    i = copy_rr[0]
                    copy_rr[0] += 1
                    if i % 2 == 0:
                        nc.scalar.copy(ot[:, half], op)
                    else:
                        nc.vector.tensor_copy(ot[:, half], op)
                    if eo % 2 == 1:
                        ring = nc.sync if (c + st) % 2 == 0 else nc.gpsimd
                        ring.dma_start(out[c, st, ep], ot)

                def pop_fillers(k):
                    for _ in range(k):
                        if filler:
                            filler.popleft()()

                def emit_attn(c):
                    nj = 4 * c + 4
                    npair = nj // 2
                    qbase = c * FD
                    ctxps = {
                        h: ps_ctx.tile([P, FD], f32, tag=f"ctx{h}",
                                       name=f"ctx_{c}_{h}")
                        for h in (0, 1)
                    }
                    acc2 = {
                        h: accp.tile([P, 2 * FD], f16, tag=f"acc{h}",
                                     name=f"acc_{c}_{h}")
                        for h in (0, 1)
                    }
                    pt_hist = {}

                    def emit_C(J):
                        for h in (0, 1):
                            pt2 = pt_hist[(h, J)]
                            for t in (0, 1):
                                j = 2 * J + t
                                o = j - 4 * c
                                so = DIAG_START[o] if o >= 0 else 0
                                nc.tensor.matmul(
                                    ctxps[h][:, so:],
                                    lhsT=vn[:, j, :],
                                    rhs=pt2[:, t * FD + so:(t + 1) * FD],
                                    start=(j == 0), stop=(j == nj - 1),
                                )

                    for J in range(npair):
                        diagA = J == npair - 2
                        diagB = J == npair - 1
                        for h in (0, 1):
                            sp2 = ps_s.tile([P, 2 * FD], f32, tag=f"s{h}",
                                            name=f"s_{c}_{h}_{J}")
                            for t in (0, 1):
                                j = 2 * J + t
                                o = j - 4 * c
                                so = DIAG_START[o] if o >= 0 else 0
                                nc.tensor.matmul(
                                    sp2[:, t * FD + so:(t + 1) * FD],
                                    lhsT=kr[:, j * P:(j + 1) * P],
                                    rhs=qr[:, h, qbase + so:qbase + FD],
                                    start=True, stop=True,
                                )
                            pt2 = ptp.tile([P, 2 * FD], f16, tag=f"pt{h}",
                                           name=f"pt_{c}_{h}_{J}")
                            if diagA:
                                rngs = ((0, FD), (FD + 128, 2 * FD))
                            elif diagB:
                                rngs = ((256, FD), (FD + 384, 2 * FD))
                            else:
                                rngs = ((0, 2 * FD),)
                            for a, b in rngs:
                                nc.scalar.activation(pt2[:, a:b], sp2[:, a:b],
                                                     Exp)
                            if diagA:
                                nc.vector.tensor_mul(
                                    pt2[:, 0:128], pt2[:, 0:128], mk)
                                nc.vector.tensor_mul(
                                    pt2[:, FD + 128:FD + 256],
                                    pt2[:, FD + 128:FD + 256], mk)
                            if diagB:
                                nc.vector.tensor_mul(
                                    pt2[:, 256:384], pt2[:, 256:384], mk)
                                nc.vector.tensor_mul(
                                    pt2[:, FD + 384:2 * FD],
                                    pt2[:, FD + 384:2 * FD], mk)
                            if J == 0:
                                if c == 0:
                                    nc.vector.memset(
                                        acc2[h][:, FD:FD + 128], 0.0)
                                    for a, b in rngs:
                                        nc.vector.tensor_copy(
                                            acc2[h][:, a:b], pt2[:, a:b])
                                else:
                                    nc.vector.tensor_copy(acc2[h], pt2)
                            else:
                                for a, b in rngs:
                                    nc.vector.tensor_add(
                                        acc2[h][:, a:b], acc2[h][:, a:b],
                                        pt2[:, a:b])
                            pt_hist[(h, J)] = pt2
                        if J >= 1:
                            emit_C(J - 1)
                        pop_fillers(3)
                    emit_C(npair - 1)
                    # per-head softmax tail
                    for h in (0, 1):
                        rp_t = ps_o.tile([P, FD], f32, tag="dop",
                                         name=f"rp_{c}_{h}")
                        rp = rp_t[0:1, 0:FD]
                        nc.tensor.matmul(rp, lhsT=on, rhs=acc2[h][:, :FD],
                                         start=True, stop=False)
                        nc.tensor.matmul(rp, lhsT=on, rhs=acc2[h][:, FD:],
                                         start=False, stop=True)
                        rec = rbp.tile([1, FD], f32, tag="rec",
                                       name=f"rec_{c}_{h}")
                        nc.vector.reciprocal_approx_fast(rec, rp)
                        rb = rbp.tile([P, FD], f32, tag="rb",
                                      name=f"rb_{c}_{h}")
                        nc.gpsimd.partition_broadcast(rb, rec)
                        ct = ctp.tile([P, FD], f16, tag=f"ct{h}",
                                      name=f"ct_{c}_{h}")
                        nc.vector.tensor_mul(ct, ctxps[h], rb)
                        ct_tiles[(c, h)] = ct
                        pop_fillers(1)

                for c in range(NSC):
                    emit_attn(c)
                    for st in range(4):
                        for eo in range(4):
                            filler.append(
                                lambda c=c, st=st, eo=eo:
                                emit_dense_unit(c, st, eo))
                while filler:
                    filler.popleft()()
    nc.compile()
    return nc


def make_in_maps(x, w_qkv, w_dense):
    x = np.asarray(x, np.float32).reshape(S, E)
    w_qkv = np.asarray(w_qkv, np.float32)
    w_dense = np.asarray(w_dense, np.float32)
    xTt = np.ascontiguousarray(x.T.reshape(NE, P, S)).astype(np.float16)
    consts = _host_constants()
    in_maps = []
    scale = np.float32(1.0 / np.sqrt(D))
    for d in range(NCORES):
        g = d // 2
        wq = w_qkv[2 * d * P:(2 * d + 2) * P] * scale
        wk = w_qkv[H * D + g * P: H * D + (g + 1) * P]
        wv = w_qkv[H * D + KVH * D + g * P: H * D + KVH * D + (g + 1) * P]
        wcat = np.concatenate([wq, wk, wv], 0)          # [FLOC, E]
        # [4-eo-group, p, eo-in-group, floc]: 4KB contiguous per partition
        wqkvT_d = np.ascontiguousarray(
            wcat.T.reshape(4, 4, P, FLOC).transpose(0, 2, 1, 3)
        ).astype(np.float16)
        wdT_d = np.ascontiguousarray(
            w_dense[:, 2 * d * P:(2 * d + 2) * P].T
            .reshape(2, P, E).transpose(1, 0, 2)).astype(np.float16)
        m = {"xTt": xTt, "wqkvT": wqkvT_d, "wdT": wdT_d}
        m.update(consts)
        in_maps.append(m)
    return in_maps


def kernel(x, w_qkv, w_dense):
    global LAST_RESULT, _BASS_CACHE
    from concourse.bass_utils import run_bass_kernel_spmd

    in_maps = make_in_maps(x, w_qkv, w_dense)
    if _BASS_CACHE is None:
        _BASS_CACHE = _build_bass()
    res = run_bass_kernel_spmd(_BASS_CACHE, in_maps, core_ids=list(range(NCORES)))
    LAST_RESULT = res
    # sum partials over cores, then untile [c, st, eo, p, f] -> [s, e]
    acc = np.zeros((NSC, 4, 2, P, 2 * FD), np.float32)
    for r in res.results:
        acc += r["out"].astype(np.float32)
    full = acc.transpose(0, 1, 3, 2, 4).reshape(S, E)
    return np.ascontiguousarray(full).reshape(B, S, E)


# revision 11
# speedup vs baseline: 1.2932x; 1.0103x over previous
"""Tensor-parallel MultiHeadAttention (QKV + RoPE + GQA causal SDPA + dense)
for 8 Trainium2 NeuronCores.

Sharding (TP): core d owns query heads {2d, 2d+1} and kv head d//2 (kv heads
replicated across core pairs), plus the matching 256 rows of w_dense. Each
core produces a full-shape [S, E] partial; the all-reduce is a host-side sum
over the 8 fp16 partials.

All matmul operands are fp16 (1.0 PE cycles/row, same as bf16; half the DMA
bytes of the old float32r kernel); PSUM accumulation stays fp32, so the end
to-end rel-err is ~1e-3 against the 2e-2 gate.

Per-core pipeline, two phases:

Phase A (QKV + RoPE + v transpose), sequential over the 4 fused-row blocks
(q0, q1, k, v) per 512-query chunk so each block's 16-matmul contraction
chain owns one PSUM bank and the post-processing of block f overlaps the
chain of block f+1 (rot matmuls are emitted one chain late to hide the
PSUM->SBUF copy latency from the in-order PE queue):
  qkv^T = W_f^T x^T -> [128, 512] ; RoPE via permutation matmul + DVE
  combine (all-fp16 DVE ops take the 2x path); v transposed 128x128 via PE.

Phase B (attention + dense), per 512-query chunk. Key tiles are processed in
PAIRS: the two S^T = k^T q matmuls of a pair land in one 2-bank PSUM tile so
one ScalarE exp covers 1024 columns (ScalarE is the attention-phase
bottleneck; wider activations amortize its ~217ns/instr overhead). The two
heads' streams interleave and ctx matmuls lag one pair-step behind the S
matmuls, so the in-order PE queue never waits on an exp. Diagonal pairs
compute/exp only their causally visible column ranges (no wasted exp work,
no stale-PSUM reads); the partial 128-wide triangles are masked
multiplicatively after exp. Softmax denominators accumulate on DVE in fp16,
are column-summed with a ones-vector matmul, reciprocal'd, and broadcast.
Dense matmuls of chunk c-1 are interleaved as PE filler inside chunk c's
attention steps; their PSUM->SBUF fp16 drains round-robin over ScalarE/DVE
(GPSIMD and DMA have no PSUM route).
"""

import collections

import numpy as np

B, S, E = 1, 2048, 2048
H, KVH, D = 16, 4, 128
NCORES = 8
P = 128
FD = 512            # matmul moving free dim == one fp32 PSUM bank
NE = E // P         # 16 contraction tiles over the embedding dim
NSC = S // FD       # 4 sequence chunks
NST = S // P        # 16 sequence tiles
FLOC = 4 * P        # local fused qkv rows per core (2 q heads + k + v)
ROPE_BASE = 10000.0
# causally visible query sub-range start for diagonal sk tile o
DIAG_START = (0, 128, 256, 384)

LAST_RESULT = None
_BASS_CACHE = None


def _rope_tables():
    inv = 1.0 / (ROPE_BASE ** (np.arange(0, D, 2, dtype=np.float64) / D))
    t = np.arange(S, dtype=np.float64)
    freqs = np.outer(t, inv)
    emb = np.concatenate([freqs, freqs], axis=-1)  # [S, D]
    return np.cos(emb), np.sin(emb)


def _host_constants():
    cos, sin = _rope_tables()
    consts = {}
    consts["cosr"] = np.ascontiguousarray(cos.T).astype(np.float16)
    consts["sinr"] = np.ascontiguousarray(sin.T).astype(np.float16)
    r_idx = np.arange(P)[:, None]
    c_idx = np.arange(P)[None, :]
    consts["tri"] = (r_idx <= c_idx).astype(np.float16)
    # rotate_half as a matmul: rot = M @ q (in [d, s] layout); pass M.T as lhsT
    M = np.zeros((P, P), np.float16)
    half = D // 2
    M[np.arange(half), np.arange(half) + half] = -1.0
    M[np.arange(half) + half, np.arange(half)] = 1.0
    consts["protT"] = np.ascontiguousarray(M.T)
    consts["ident"] = np.eye(P, dtype=np.float16)
    consts["ones"] = np.ones((P, 1), np.float16)
    return consts


def _build_bass():
    import concourse.mybir as mybir
    import concourse.tile as tile
    from concourse import bacc

    f32 = mybir.dt.float32
    f16 = mybir.dt.float16
    Exp = mybir.ActivationFunctionType.Exp

    nc = bacc.Bacc(None, target_bir_lowering=False, name="mha_tp8v2")
    xTt = nc.dram_tensor("xTt", [NE, P, S], f16, kind="ExternalInput")
    wqkvT = nc.dram_tensor("wqkvT", [4, P, 4, FLOC], f16, kind="ExternalInput")
    wdT = nc.dram_tensor("wdT", [P, 2, S], f16, kind="ExternalInput")
    cosr = nc.dram_tensor("cosr", [P, S], f16, kind="ExternalInput")
    sinr = nc.dram_tensor("sinr", [P, S], f16, kind="ExternalInput")
    trid = nc.dram_tensor("tri", [P, P], f16, kind="ExternalInput")
    protT = nc.dram_tensor("protT", [P, P], f16, kind="ExternalInput")
    ident = nc.dram_tensor("ident", [P, P], f16, kind="ExternalInput")
    ones = nc.dram_tensor("ones", [P, 1], f16, kind="ExternalInput")
    # output tiled [c, st, eo, p, f]; host reassembles to [s, e]
    out = nc.dram_tensor("out", [NSC, 4, 2, P, 2 * FD], f16, kind="ExternalOutput")

    with tile.TileContext(nc) as tc:
        with tc.tile_pool(name="const", bufs=1) as const:
            xs = const.tile([P, NE, S], f16, name="xs")
            w_sb = const.tile([P, NE, FLOC], f16, name="w_sb")
            wd_sb = const.tile([P, 2, S], f16, name="wd_sb")
            cq = const.tile([P, S], f16, name="cq")
            sq_t = const.tile([P, S], f16, name="sq_t")
            mk = const.tile([P, P], f16, name="mk")
            pr = const.tile([P, P], f16, name="pr")
            idn = const.tile([P, P], f16, name="idn")
            on = const.tile([P, 1], f16, name="on")

            junk = const.tile([P, P], f16, name="junk")
            qr = const.tile([P, 2, S], f16, name="qr")
            kr = const.tile([P, S], f16, name="kr")
            vn = const.tile([P, NST, P], f16, name="vn")

            # ---- Phase A: fused QKV projection + RoPE + v transpose ----
            with tc.tile_pool(name="pq", bufs=1, space="PSUM") as pqkv, \
                 tc.tile_pool(name="prt", bufs=2, space="PSUM") as prot_p, \
                 tc.tile_pool(name="pv", bufs=1, space="PSUM") as pvt, \
                 tc.tile_pool(name="pw", bufs=1, space="PSUM") as pwarm, \
                 tc.tile_pool(name="rtmp", bufs=4) as rtmp, \
                 tc.tile_pool(name="vtp", bufs=2) as vtp:
                # PE warm-up on a memset tile (no DMA dependency): ramps the
                # Tensor engine to max p-state while weights/x are in flight
                nc.vector.memset(junk, 1.0)
                warm = pwarm.tile([1, P], f32, tag="warm", name="warm")
                for _ in range(40):
                    nc.tensor.matmul(warm, lhsT=junk[:, 0:1], rhs=junk,
                                     start=True, stop=True)
                # consts on the gpsimd SWDGE ring (not start-critical)
                nc.gpsimd.dma_start(pr, protT[:, :])
                nc.gpsimd.dma_start(cq, cosr[:, :])
                nc.gpsimd.dma_start(sq_t, sinr[:, :])
                nc.gpsimd.dma_start(idn, ident[:, :])
                nc.gpsimd.dma_start(on, ones[:, :])
                nc.gpsimd.dma_start(mk, trid[:, :])
                nc.gpsimd.dma_start(wd_sb, wdT[:, :, :])
                # w first (4-eo groups, 4KB descriptors), then per-eo x slabs
                # (4KB descriptors) 3-way across the sync/scalar/gpsimd rings:
                # ring descriptor GENERATION (~8ns/desc) is the real input
                # pacer, so balance descriptor count, not bytes
                nc.sync.dma_start(w_sb[:, 0:4, :], wqkvT[0])
                nc.scalar.dma_start(w_sb[:, 4:8, :], wqkvT[1])
                nc.sync.dma_start(w_sb[:, 8:12, :], wqkvT[2])
                nc.scalar.dma_start(w_sb[:, 12:16, :], wqkvT[3])
                # two-stage x delivery: chunk-0 columns for every eo first
                # (the first QKV chain touches all 16 eo tiles within ~14us),
                # then the chunk-1..3 remainders as 3KB-line slabs
                for eo in range(NE):
                    ring = nc.sync if eo % 2 == 0 else nc.scalar
                    ring.dma_start(xs[:, eo, 0:FD], xTt[eo, :, 0:FD])
                for eo in range(NE):
                    ring = nc.sync if eo % 2 == 0 else nc.scalar
                    ring.dma_start(xs[:, eo, FD:], xTt[eo, :, FD:])

                def make_rope(c, f, qt, ssl):
                    def emit():
                        rps = prot_p.tile([P, FD], f32, tag="rot",
                                          name=f"rot_{c}_{f}")
                        nc.tensor.matmul(rps, lhsT=pr, rhs=qt,
                                         start=True, stop=True)
                        tt = rtmp.tile([P, FD], f16, tag="tt",
                                       name=f"tt_{c}_{f}")
                        nc.vector.tensor_mul(tt, rps, sq_t[:, ssl])
                        dst = qr[:, f, ssl] if f < 2 else kr[:, ssl]
                        nc.vector.tensor_mul(dst, qt, cq[:, ssl])
                        nc.vector.tensor_add(dst, dst, tt)
                    return emit

                def make_vt(c, vtt):
                    def emit():
                        vp = pvt.tile([P, 4, P], f16, tag="vt",
                                      name=f"vt_{c}")
                        for jj in range(4):
                            nc.tensor.transpose(
                                vp[:, jj, :], vtt[:, jj * P:(jj + 1) * P],
                                idn)
                        nc.scalar.copy(vn[:, 4 * c:4 * c + 4, :], vp)
                    return emit

                pend = collections.deque()
                for c in range(NSC):
                    ssl = slice(c * FD, (c + 1) * FD)
                    psums = [
                        pqkv.tile([P, FD], f32, tag=f"qkv{f}",
                                  name=f"qkv_{c}_{f}")
                        for f in range(4)
                    ]
                    for eo in range(NE):
                        for f in range(4):
                            nc.tensor.matmul(
                                psums[f],
                                lhsT=w_sb[:, eo, f * P:(f + 1) * P],
                                rhs=xs[:, eo, ssl],
                                start=(eo == 0),
                                stop=(eo == NE - 1),
                            )
                        # previous chunk's rope/vt PE work interleaves here
                        # (the eo==0 pop also hides the qkv-psum copy wait at
                        # each chunk boundary)
                        if (eo == 0 or eo % 4 == 3) and pend:
                            pend.popleft()()
                    for f in range(4):
                        if f < 3:
                            qt = rtmp.tile([P, FD], f16, tag=f"qt{f}",
                                           name=f"qt_{c}_{f}")
                            nc.scalar.copy(qt, psums[f])
                            pend.append(make_rope(c, f, qt, ssl))
                        else:
                            vtt = vtp.tile([P, FD], f16, tag="vT",
                                           name=f"vT_{c}")
                            nc.scalar.copy(vtt, psums[f])
                            pend.append(make_vt(c, vtt))
                while pend:
                    pend.popleft()()

            # ---- Phase B: attention + dense, per 512-query chunk ----
            with tc.tile_pool(name="ps_s", bufs=1, space="PSUM") as ps_s, \
                 tc.tile_pool(name="ps_ctx", bufs=1, space="PSUM") as ps_ctx, \
                 tc.tile_pool(name="ps_o", bufs=2, space="PSUM") as ps_o, \
                 tc.tile_pool(name="ptp", bufs=3) as ptp, \
                 tc.tile_pool(name="accp", bufs=2) as accp, \
                 tc.tile_pool(name="rbp", bufs=2) as rbp, \
                 tc.tile_pool(name="ctp", bufs=2) as ctp, \
                 tc.tile_pool(name="outp", bufs=6) as outp:

                ct_tiles = {}
                filler = collections.deque()
                copy_rr = [0]

                ot_hold = {}

                def emit_dense_unit(c, st, eo):
                    op = ps_o.tile([P, FD], f32, tag="dop",
                                   name=f"dop_{c}_{st}_{eo}")
                    nc.tensor.matmul(
                        op, lhsT=ct_tiles[(c, 0)][:, st * P:(st + 1) * P],
                        rhs=wd_sb[:, 0, eo * FD:(eo + 1) * FD],
                        start=True, stop=False)
                    nc.tensor.matmul(
                        op, lhsT=ct_tiles[(c, 1)][:, st * P:(st + 1) * P],
                        rhs=wd_sb[:, 1, eo * FD:(eo + 1) * FD],
                        start=False, stop=True)
                    # eo pairs share one [P, 1024] staging tile so the out
                    # DMA moves 2KB per partition line (ring-gen efficiency)
                    ep = eo // 2
                    if eo % 2 == 0:
                        ot_hold[(c, st, ep)] = outp.tile(
                            [P, 2 * FD], f16, tag="ot",
                            name=f"ot_{c}_{st}_{ep}")
                    ot = ot_hold[(c, st, ep)]
                    half = slice((eo % 2) * FD, (eo % 2) * FD + FD)
                    i = copy_rr[0]
                    copy_rr[0] += 1
                    if i % 2 == 0:
                        nc.scalar.copy(ot[:, half], op)
                    else:
                        nc.vector.tensor_copy(ot[:, half], op)
                    if eo % 2 == 1:
                        ring = nc.sync if (c + st) % 2 == 0 else nc.gpsimd
                        ring.dma_start(out[c, st, ep], ot)

                def pop_fillers(k):
                    for _ in range(k):
                        if filler:
                            filler.popleft()()

                def emit_attn(c):
                    nj = 4 * c + 4
                    npair = nj // 2
                    qbase = c * FD
                    ctxps = {
                        h: ps_ctx.tile([P, FD], f32, tag=f"ctx{h}",
                                       name=f"ctx_{c}_{h}")
                        for h in (0, 1)
                    }
                    acc2 = {
                        h: accp.tile([P, 2 * FD], f16, tag=f"acc{h}",
                                     name=f"acc_{c}_{h}")
                        for h in (0, 1)
                    }
                    pt_hist = {}

                    def emit_C(J):
                        for h in (0, 1):
                            pt2 = pt_hist[(h, J)]
                            for t in (0, 1):
                                j = 2 * J + t
                                o = j - 4 * c
                                so = DIAG_START[o] if o >= 0 else 0
                                nc.tensor.matmul(
                                    ctxps[h][:, so:],
                                    lhsT=vn[:, j, :],
                                    rhs=pt2[:, t * FD + so:(t + 1) * FD],
                                    start=(j == 0), stop=(j == nj - 1),
                                )

                    for J in range(npair):
                        diagA = J == npair - 2
                        diagB = J == npair - 1
                        for h in (0, 1):
                            sp2 = ps_s.tile([P, 2 * FD], f32, tag=f"s{h}",
                                            name=f"s_{c}_{h}_{J}")
                            for t in (0, 1):
                                j = 2 * J + t
                                o = j - 4 * c
                                so = DIAG_START[o] if o >= 0 else 0
                                nc.tensor.matmul(
                                    sp2[:, t * FD + so:(t + 1) * FD],
                                    lhsT=kr[:, j * P:(j + 1) * P],
                                    rhs=qr[:, h, qbase + so:qbase + FD],
                                    start=True, stop=True,
                                )
                            pt2 = ptp.tile([P, 2 * FD], f16, tag=f"pt{h}",
                                           name=f"pt_{c}_{h}_{J}")
                            if diagA:
                                rngs = ((0, FD), (FD + 128, 2 * FD))
                            elif diagB:
                                rngs = ((256, FD), (FD + 384, 2 * FD))
                            else:
                                rngs = ((0, 2 * FD),)
                            for a, b in rngs:
                                nc.scalar.activation(pt2[:, a:b], sp2[:, a:b],
                                                     Exp)
                            if diagA:
                                nc.vector.tensor_mul(
                                    pt2[:, 0:128], pt2[:, 0:128], mk)
                                nc.vector.tensor_mul(
                                    pt2[:, FD + 128:FD + 256],
                                    pt2[:, FD + 128:FD + 256], mk)
                            if diagB:
                                nc.vector.tensor_mul(
                                    pt2[:, 256:384], pt2[:, 256:384], mk)
                                nc.vector.tensor_mul(
                                    pt2[:, FD + 384:2 * FD],
                                    pt2[:, FD + 384:2 * FD], mk)
                            if J == 0:
                                if c == 0:
                                    nc.vector.memset(
                                        acc2[h][:, FD:FD + 128], 0.0)
                                    for a, b in rngs:
                                        nc.vector.tensor_copy(
                                            acc2[h][:, a:b], pt2[:, a:b])
                                else:
                                    nc.vector.tensor_copy(acc2[h], pt2)
                            else:
                                for a, b in rngs:
                                    nc.vector.tensor_add(
                                        acc2[h][:, a:b], acc2[h][:, a:b],
                                        pt2[:, a:b])
                            pt_hist[(h, J)] = pt2
                        if J >= 1:
                            emit_C(J - 1)
                        pop_fillers(3)
                    emit_C(npair - 1)
                    # per-head softmax tail
                    for h in (0, 1):
                        rp_t = ps_o.tile([P, FD], f32, tag="dop",
                                         name=f"rp_{c}_{h}")
                        rp = rp_t[0:1, 0:FD]
                        nc.tensor.matmul(rp, lhsT=on, rhs=acc2[h][:, :FD],
                                         start=True, stop=False)
                        nc.tensor.matmul(rp, lhsT=on, rhs=acc2[h][:, FD:],
                                         start=False, stop=True)
                        rec = rbp.tile([1, FD], f32, tag="rec",
                                       name=f"rec_{c}_{h}")
                        nc.vector.reciprocal_approx_fast(rec, rp)
                        rb = rbp.tile([P, FD], f32, tag="rb",
                                      name=f"rb_{c}_{h}")
                        nc.gpsimd.partition_broadcast(rb, rec)
                        ct = ctp.tile([P, FD], f16, tag=f"ct{h}",
                                      name=f"ct_{c}_{h}")
                        nc.vector.tensor_mul(ct, ctxps[h], rb)
                        ct_tiles[(c, h)] = ct
                        pop_fillers(1)

                for c in range(NSC):
                    emit_attn(c)
                    for st in range(4):
                        for eo in range(4):
                            filler.append(
                                lambda c=c, st=st, eo=eo:
                                emit_dense_unit(c, st, eo))
                while filler:
                    filler.popleft()()
    nc.compile()
    return nc


def make_in_maps(x, w_qkv, w_dense):
    x = np.asarray(x, np.float32).reshape(S, E)
    w_qkv = np.asarray(w_qkv, np.float32)
    w_dense = np.asarray(w_dense, np.float32)
    xTt = np.ascontiguousarray(x.T.reshape(NE, P, S)).astype(np.float16)
    consts = _host_constants()
    in_maps = []
    scale = np.float32(1.0 / np.sqrt(D))
    for d in range(NCORES):
        g = d // 2
        wq = w_qkv[2 * d * P:(2 * d + 2) * P] * scale
        wk = w_qkv[H * D + g * P: H * D + (g + 1) * P]
        wv = w_qkv[H * D + KVH * D + g * P: H * D + KVH * D + (g + 1) * P]
        wcat = np.concatenate([wq, wk, wv], 0)          # [FLOC, E]
        # [4-eo-group, p, eo-in-group, floc]: 4KB contiguous per partition
        wqkvT_d = np.ascontiguousarray(
            wcat.T.reshape(4, 4, P, FLOC).transpose(0, 2, 1, 3)
        ).astype(np.float16)
        wdT_d = np.ascontiguousarray(
            w_dense[:, 2 * d * P:(2 * d + 2) * P].T
            .reshape(2, P, E).transpose(1, 0, 2)).astype(np.float16)
        m = {"xTt": xTt, "wqkvT": wqkvT_d, "wdT": wdT_d}
        m.update(consts)
        in_maps.append(m)
    return in_maps


def kernel(x, w_qkv, w_dense):
    global LAST_RESULT, _BASS_CACHE
    from concourse.bass_utils import run_bass_kernel_spmd

    in_maps = make_in_maps(x, w_qkv, w_dense)
    if _BASS_CACHE is None:
        _BASS_CACHE = _build_bass()
    res = run_bass_kernel_spmd(_BASS_CACHE, in_maps, core_ids=list(range(NCORES)))
    LAST_RESULT = res
    # sum partials over cores, then untile [c, st, eo, p, f] -> [s, e]
    acc = np.zeros((NSC, 4, 2, P, 2 * FD), np.float32)
    for r in res.results:
        acc += r["out"].astype(np.float32)
    full = acc.transpose(0, 1, 3, 2, 4).reshape(S, E)
    return np.ascontiguousarray(full).reshape(B, S, E)
